# revision 1
# baseline (speedup 1.0000x reference)
"""TRN2 Bass/Tile kernel for the cosine-similarity attention block.

Reference math (fp32, single device):
    K = X @ Wk.T + Wk0 ; Q = X @ Wq.T + Wq0          # [N, E]
    Y = (Q @ K.T) / sqrt(max(|Q_m|^2 * |K_n|^2, eps)) # [N, N] cosine sims
    SM = softmax(Y, axis=0)                           # column softmax
    Z = SM @ X                                        # [N, E]

Distribution (8 cores, row-sharded): each core owns M = N/8 rows of Q /
output rows of Z. Everything heavy runs as fp8-e4m3 DoubleRow matmuls
(2 k-tiles per pass at 0.5 cyc/row; fp32 PSUM accumulation):

  Phase 0: K/Q projections from fp8 X^T/W^T; row norms via squares +
    ones-matmul reduction. K ships UNNORMALIZED (x4 fp8) straight off
    the projection -- its AllGather launches as soon as the last K tile
    is projected -- while 1/(16|k_n|) travels as a tiny second fp16
    AllGather and is folded into the exp's per-partition scale (XBAR
    DMA-transposed into [128, nt] on the reader). Only Q pays the
    broadcast-normalize (x4/|q| via ones-matmul + DVE scale).
  Phase 2: Yt slice via DoubleRow (psy holds 16|k_n|*y), exp on ScalarE
    with scale=1/(16|k_n|) and accum_out producing per-column partial
    sums for free (fp16 ring buffer). Column sums AllReduce in 4 chunks
    of 16 n-tiles so early chunks overlap the Y phase.
  Phase 3 (per AR chunk): v = (exp(y) - cs/8192) * (2^17/cs), i.e.
    2^17*(SM - 1/8192), folded on DVE (+ScalarE for the last chunk)
    into fp8 pair tiles. Centering before the fp8 quantization is what
    makes fp8 viable: softmax weights concentrate at 1/8192*(1 +- 0.12),
    so raw e4m3 (3 mantissa bits) would destroy the signal, while the
    centered residuals span binades. (GPSIMD is software-emulated on
    TRN2 -- never put bulk elementwise work there.)
  Phase 4: Zt = X^T-panels(fp8) @ V(fp8) via DoubleRow, then
    Zt = psz/2^17 + Sx/8192 with Sx = colsum of X (host-exact),
    restoring the centering: SM@X = (SM-c)@X + c*(1^T X), c = 1/8192.

The repeat loop covers all phases (per-rep AG/AR buffers) so a
repeat-NEFF slope measures honest steady-state per-exec device time.
Measured end-to-end error ~7.4e-3 scale-relative absmax (gate: 2e-2).
"""

from contextlib import ExitStack

import numpy as np

N, E, C = 8192, 1024, 8

_CACHE = {}


def _build_program(n=N, e=E, c=C, solo=False, repeat=1, upto=4):
    """Emit + compile the SPMD Bass program (one NEFF, all cores)."""
    import concourse.bacc as bacc
    import concourse.mybir as mybir
    import concourse.tile as tile

    F32 = mybir.dt.float32
    F16 = mybir.dt.float16
    F8 = mybir.dt.float8e4
    AF = mybir.ActivationFunctionType
    PM = mybir.MatmulPerfMode.DoubleRow
    ALU = mybir.AluOpType

    m = n // c          # rows per core
    et = e // 128       # e-tiles
    nt = n // 128       # n-tiles
    jt = m // 128       # n-tiles per core block
    mch = [(i, min(512, m - i)) for i in range(0, m, 512)]  # m chunks (<=512)
    CH = 16             # AR chunk size in n-tiles
    nch = nt // CH
    EB = 32             # fp16 exp ring size (tiles)
    S_V = 131072.0      # 2^17: fp8 scale for centered softmax weights
    rg = [list(range(c))]

    nc = bacc.Bacc("TRN2", target_bir_lowering=False, debug=False, num_devices=c)

    xt = nc.dram_tensor("xt", [e, m], F8, kind="ExternalInput")
    wqt = nc.dram_tensor("wqt", [e, e], F8, kind="ExternalInput")
    wkt = nc.dram_tensor("wkt", [e, e], F8, kind="ExternalInput")
    bq = nc.dram_tensor("bq", [et, 128], F32, kind="ExternalInput")
    bk = nc.dram_tensor("bk", [et, 128], F32, kind="ExternalInput")
    xp = nc.dram_tensor("xp", [et, 128, nt, 128], F8, kind="ExternalInput")
    sx = nc.dram_tensor("sx", [et, 128], F32, kind="ExternalInput")  # X colsum/8192
    zt = nc.dram_tensor("zt", [e, m], F16, kind="ExternalOutput")

    with ExitStack() as ctx:
        tc = ctx.enter_context(tile.TileContext(nc))

        dram = ctx.enter_context(tc.tile_pool(name="dram", bufs=1, space="DRAM"))

        consts = ctx.enter_context(tc.tile_pool(name="consts", bufs=1))
        ones_k = consts.tile([128, 1], F8)
        ones4_m = consts.tile([1, 128], F16)
        nc.vector.memset(ones_k, 1.0)
        nc.vector.memset(ones4_m, 4.0)  # folds the x4 fp8 scale into Kn/Qn
        bias_q = consts.tile([128, et], F32)
        bias_k = consts.tile([128, et], F32)
        nc.sync.dma_start(bias_q, bq.ap().rearrange("t p -> p t"))
        nc.sync.dma_start(bias_k, bk.ap().rearrange("t p -> p t"))
        sxb = consts.tile([128, et], F32)
        nc.sync.dma_start(sxb, sx.ap().rearrange("t p -> p t"))
        eps1 = consts.tile([1, 1], F32)
        nc.vector.memset(eps1, 1e-6)
        eps256 = consts.tile([1, 1], F32)
        nc.vector.memset(eps256, 2.56e-4)  # 256 * 1e-6, for 16*sqrt(x+eps)
        sck16 = consts.tile([128, nt], F16)  # XBAR-transposed 1/(16|k_n|)
        sck32 = consts.tile([128, nt], F32)  # fp32 copy (scale APs must be fp32)
        colsum = consts.tile([128, nt], F32)
        cs_full = consts.tile([128, nt], F32)
        rec_cs = consts.tile([128, nt], F32)
        sfac = consts.tile([128, nt], F32)   # 2^17 / cs
        tfac = consts.tile([128, nt], F32)   # cs / 8192

        # persistent across reps: fp8 Qn^T [128, et, m], fp8 centered-SM
        # tiles in DoubleRow pair layout [128, 2, m], and the phase-0
        # working set (persistent tags let the next rep's input DMAs start
        # as soon as the previous rep's last reader is done, instead of
        # waiting for a whole pool region to free).
        qn_pool = ctx.enter_context(tc.tile_pool(name="qn", bufs=1))
        qn8 = [qn_pool.tile([128, 2, m], F8, tag=f"qn8_{p}", name=f"qn8_{p}")
               for p in range(et // 2)]
        et_pool = ctx.enter_context(tc.tile_pool(name="etp", bufs=1))
        ets8 = [et_pool.tile([128, 2, m], F8, tag=f"e8_{i}", name=f"e8_{i}")
                for i in range(nt // 2)]
        p0 = ctx.enter_context(tc.tile_pool(name="p0", bufs=1))
        x8 = [p0.tile([128, 2, m], F8, tag=f"x8_{p}", name=f"x8_{p}")
              for p in range(et // 2)]
        w8 = [p0.tile([128, 2, e], F8, tag=f"w8_{p}", name=f"w8_{p}")
              for p in range(et // 2)]

        for rep in range(repeat):
            # per-rep AG buffers: a Shared DRAM buffer may have only one
            # writing instruction, so each rep's AllGather gets its own.
            ag_in = dram.tile([e, m], F8, tag=f"agi{rep}", name=f"agi{rep}")
            ag_out = dram.tile([c, e, m], F8, addr_space="Shared",
                               tag=f"ago{rep}", name=f"ago{rep}")
            rk_in = dram.tile([m], F16, tag=f"rki{rep}", name=f"rki{rep}")
            rk_out = dram.tile([c, m], F16, addr_space="Shared",
                               tag=f"rko{rep}", name=f"rko{rep}")

            # ---------------- Phase 0: projections + row norms -------------
            with (
                tc.tile_pool(name="p0t", bufs=1) as p0t,
                tc.tile_pool(name="psp", bufs=2, space="PSUM") as psp,
                tc.tile_pool(name="pss", bufs=1, space="PSUM") as pss,
            ):
                for s in range(et):
                    nc.sync.dma_start(x8[s // 2][:, s % 2, :],
                                      xt.ap()[s * 128:(s + 1) * 128, :])

                def load_w(wbuf, w_handle):
                    for s in range(et):
                        nc.sync.dma_start(
                            wbuf[s // 2][:, s % 2, :],
                            w_handle.ap()[s * 128:(s + 1) * 128, :],
                        )

                # separate pf/sq sets per projection: lets Q's matmuls emit
                # before K's norm chain without a WAR on K's pf reads. They
                # live in the per-rep pool so etsA can reuse the space later.
                pfs = {nm: [p0t.tile([128, m], F16, tag=f"pf{nm}{t}",
                                     name=f"pf{nm}{t}") for t in range(et)]
                       for nm in "kq"}
                sqs = {nm: [p0t.tile([128, m], F8, tag=f"sq{nm}{t}",
                                     name=f"sq{nm}{t}") for t in range(et)]
                       for nm in "kq"}

                def proj_mm(nm, wbuf, bias_sb, post_t=None):
                    pf, sq = pfs[nm], sqs[nm]
                    for t in range(et):
                        ps = psp.tile([128, m], F32, tag="pp", name="proj_ps")
                        for sp in range(et // 2):
                            lw = wbuf[sp][:, :, t * 128:(t + 1) * 128]
                            for o, w in mch:
                                nc.tensor.matmul(
                                    ps[:, o:o + w],
                                    lw,
                                    x8[sp][:, :, o:o + w],
                                    start=(sp == 0),
                                    stop=(sp == et // 2 - 1),
                                    perf_mode=PM,
                                )
                        nc.scalar.activation(pf[t], ps, AF.Identity,
                                             bias=bias_sb[:, t:t + 1])
                        nc.vector.tensor_mul(sq[t], pf[t], pf[t])
                        if post_t is not None:
                            post_t(t)

                def d_ps_reduce(nm):
                    d_ps = pss.tile([1, m], F32, tag="dps", name="d_ps")
                    for o, w in mch:
                        for t in range(et):
                            nc.tensor.matmul(
                                d_ps[0:1, o:o + w],
                                ones_k,
                                sqs[nm][t][:, o:o + w],
                                start=(t == 0),
                                stop=(t == et - 1),
                            )
                    return d_ps

                def norm_chain(nm, d_ps, out8, dram_out):
                    # m-chunked so the sqrt -> recip -> broadcast -> scale
                    # chain pipelines per chunk.
                    pf = pfs[nm]
                    bc_ps = pss.tile([128, m], F32, tag="bc", name="bc_ps")
                    dsq = p0t.tile([1, m], F32, tag="dsq", name="dsq")
                    rnorm = p0t.tile([1, m], F32, tag="rn", name="rnorm")
                    rn16 = p0t.tile([1, m], F16, tag="rn16", name="rn16")
                    for o, w in mch:
                        nc.scalar.activation(dsq[0:1, o:o + w], d_ps[0:1, o:o + w],
                                             AF.Sqrt, bias=eps1[0:1, 0:1])
                        nc.vector.reciprocal(rnorm[0:1, o:o + w], dsq[0:1, o:o + w])
                        nc.vector.tensor_copy(rn16[0:1, o:o + w],
                                              rnorm[0:1, o:o + w])
                        nc.tensor.matmul(
                            bc_ps[:, o:o + w],
                            ones4_m,
                            rn16[0:1, o:o + w],
                        )
                        for t in range(et):
                            nc.vector.tensor_mul(out8(t, o, w), bc_ps[:, o:o + w],
                                                 pf[t][:, o:o + w])
                            if dram_out is not None:
                                nc.sync.dma_start(
                                    dram_out[t * 128:(t + 1) * 128, o:o + w],
                                    out8(t, o, w),
                                )

                # K is shipped UNNORMALIZED (x4, fp8) straight off the
                # projection -- 1/(16|k_n|) is folded into the exp's
                # per-partition scale instead, so the AllGather launches as
                # soon as the last K tile is projected, and K's broadcast/
                # normalize DVE work disappears.
                kst = [p0t.tile([128, m], F8, tag=f"kst{t % 2}", name=f"kst{t % 2}")
                       for t in range(2)]

                def k_ship(t):
                    nc.scalar.activation(kst[t % 2], pfs["k"][t], AF.Copy,
                                         scale=4.0)
                    nc.sync.dma_start(ag_in[t * 128:(t + 1) * 128, :],
                                      kst[t % 2])

                load_w(w8, wkt)
                proj_mm("k", w8, bias_k, post_t=k_ship)
                if not solo:
                    nc.gpsimd.collective_compute(
                        "AllGather",
                        mybir.AluOpType.bypass,
                        replica_groups=rg,
                        ins=[ag_in.opt()],
                        outs=[ag_out.opt()],
                    )
                load_w(w8, wqt)
                # rk = 1/(16|k|): 16*sqrt(x+eps) in one activation
                # (scale=256 inside the sqrt).
                d_psk = d_ps_reduce("k")
                dsqk = p0t.tile([1, m], F32, tag="dsqk", name="dsqk")
                nc.scalar.activation(dsqk, d_psk, AF.Sqrt, scale=256.0,
                                     bias=eps256[0:1, 0:1])
                rkf = p0t.tile([1, m], F32, tag="rkf", name="rkf")
                nc.vector.reciprocal(rkf, dsqk)
                rk16 = p0t.tile([1, m], F16, tag="rk16", name="rk16")
                nc.vector.tensor_copy(rk16, rkf)
                nc.sync.dma_start(rk_in[:], rk16[0:1, :])
                if not solo:
                    nc.gpsimd.collective_compute(
                        "AllGather",
                        mybir.AluOpType.bypass,
                        replica_groups=rg,
                        ins=[rk_in.opt()],
                        outs=[rk_out.opt()],
                    )
                proj_mm("q", w8, bias_q)
                norm_chain("q", d_ps_reduce("q"),
                           lambda t, o, w: qn8[t // 2][:, t % 2, o:o + w], None)

            if upto < 2:
                continue

            with (
                tc.tile_pool(name="xpp", bufs=4) as xp_pool,
                tc.tile_pool(name="zsb", bufs=2) as z_pool,
                tc.tile_pool(name="psz", bufs=1, space="PSUM") as psz_pool,
                tc.tile_pool(name="kp", bufs=2) as kp_pool,
              ):
               xq_pre = {}

               def load_xq(t, q):
                   xq = xp_pool.tile([128, CH, 128], F8, tag="xq", name="xq")
                   nc.sync.dma_start(xq, xp.ap()[t, :, q * CH:(q + 1) * CH, :])
                   xq_pre[(t, q)] = xq
                   return xq

               def load_kp(cc):
                   kp = [kp_pool.tile([128, 2, m], F8, tag=f"kp{p}", name=f"kp{p}")
                         for p in range(et // 2)]
                   for s in range(et):
                       src_ap = (ag_in[s * 128:(s + 1) * 128, :] if solo
                                 else ag_out[cc, s * 128:(s + 1) * 128, :])
                       nc.sync.dma_start(kp[s // 2][:, s % 2, :], src_ap)
                   return kp

               # prefetch block 0's K^T before the Q-projection's norm-chain
               # work lands on the SP queue: phase 2 can then start the
               # moment qn8 is written.
               kp0 = load_kp(0)

               # exp scale tile sck16[p, i] = 1/(16|k_{i*128+p}|): transpose
               # the gathered [c*jt, 128] rk rows into partition-major via
               # the XBAR DMA (16-bit only).
               if solo:
                   nc.vector.memset(sck32, 0.0625)
               else:
                   nc.sync.dma_start_transpose(
                       sck16,
                       rk_out[:, :].rearrange("c (t p) -> (c t) p", p=128),
                   )
                   nc.vector.tensor_copy(sck32, sck16)

               with (
                tc.tile_pool(name="eA", bufs=1) as eA_pool,
                tc.tile_pool(name="psy", bufs=2, space="PSUM") as psy_pool,
               ):
                etsA = [eA_pool.tile([128, m], F16, tag=f"eA{i}", name=f"eA{i}")
                        for i in range(EB)]

                # ---- Phase 3 helper: AR one chunk of CH n-tiles, then fold
                # v = (exp(y) - cs/8192) * (2^17/cs) into fp8 pair tiles.
                def ar_chunk(k):
                    if upto < 3:
                        return
                    sl = slice(CH * k, CH * (k + 1))
                    ar_in = dram.tile([128, CH], F32, tag=f"ari{rep}_{k}",
                                      name=f"ari{rep}_{k}")
                    ar_out = dram.tile([128, CH], F32, addr_space="Shared",
                                       tag=f"aro{rep}_{k}", name=f"aro{rep}_{k}")
                    nc.sync.dma_start(ar_in, colsum[:, sl])
                    if not solo:
                        nc.gpsimd.collective_compute(
                            "AllReduce",
                            mybir.AluOpType.add,
                            replica_groups=rg,
                            ins=[ar_in.opt()],
                            outs=[ar_out.opt()],
                        )
                    nc.sync.dma_start(cs_full[:, sl], ar_in if solo else ar_out)
                    nc.vector.reciprocal(rec_cs[:, sl], cs_full[:, sl])
                    nc.vector.tensor_scalar_mul(sfac[:, sl], rec_cs[:, sl], S_V)
                    nc.vector.tensor_scalar_mul(tfac[:, sl], cs_full[:, sl],
                                                1.0 / 8192.0)
                    last = (k == nch - 1)
                    for i in range(CH * k, CH * (k + 1)):
                        dst = ets8[i // 2][:, i % 2, :]
                        src_t = etsA[i % EB]
                        # during phase 2 ScalarE is saturated with exp; only
                        # give it affine work on the last chunk. (GPSIMD is
                        # software-emulated on TRN2 -- never put bulk
                        # elementwise work there.)
                        if last and i % 2 == 1:
                            nc.scalar.activation(dst, src_t, AF.Copy,
                                                 scale=sfac[:, i:i + 1], bias=-16.0)
                        else:
                            nc.vector.tensor_scalar(
                                dst, src_t, tfac[:, i:i + 1], sfac[:, i:i + 1],
                                ALU.subtract, ALU.mult)

                # ---------------- Phase 2: Yt via fp8 DoubleRow ------------
                for cc in range(c):
                    kp = kp0 if cc == 0 else load_kp(cc)
                    for j in range(jt):
                        i = cc * jt + j
                        psy = psy_pool.tile([128, m], F32, tag="py", name="psy")
                        for sp in range(et // 2):
                            lw = kp[sp][:, :, j * 128:(j + 1) * 128]
                            for o, w in mch:
                                nc.tensor.matmul(
                                    psy[:, o:o + w],
                                    lw,
                                    qn8[sp][:, :, o:o + w],
                                    start=(sp == 0),
                                    stop=(sp == et // 2 - 1),
                                    perf_mode=PM,
                                )
                        nc.scalar.activation(
                            etsA[i % EB], psy, AF.Exp,
                            scale=sck32[:, i:i + 1],
                            accum_out=colsum[:, i:i + 1],
                        )
                        if i % CH == CH - 1:
                            ar_chunk(i // CH)
                            if i // CH == 1 and upto >= 4:
                                # prefetch phase 4's first X^T panels so its
                                # first matmuls don't wait on DMA latency.
                                load_xq(0, 0)
                                load_xq(0, 1)

               # ---------------- Phase 4: Zt = X^T @ V (fp8 DoubleRow) -----
               if upto < 4:
                    continue
               for t in range(et):
                    psz = psz_pool.tile([128, m], F32, tag=f"pz{t % 2}",
                                        name=f"pz{t % 2}")
                    for q in range(nt // CH):
                        xq = xq_pre.pop((t, q), None)
                        if xq is None:
                            xq = load_xq(t, q)
                        for p2 in range(CH // 2):
                            pair = q * (CH // 2) + p2
                            lw = xq[:, 2 * p2:2 * p2 + 2, :]
                            for o, w in mch:
                                nc.tensor.matmul(
                                    psz[:, o:o + w],
                                    lw,
                                    ets8[pair][:, :, o:o + w],
                                    start=(pair == 0),
                                    stop=(pair == nt // 2 - 1),
                                    perf_mode=PM,
                                )
                    zsb = z_pool.tile([128, m], F16, tag="zt", name="zsb")
                    nc.scalar.activation(zsb, psz, AF.Identity,
                                         scale=1.0 / S_V, bias=sxb[:, t:t + 1])
                    nc.sync.dma_start(zt.ap()[t * 128:(t + 1) * 128, :], zsb)

    nc.compile()
    return nc


def _prep_inputs(X, Wk, Wq, Wk0, Wq0, n=N, e=E, c=C):
    """Host-side sharding/layout prep. Returns per-core input maps."""
    import concourse.mybir as mybir

    f8 = mybir.dt.np(mybir.dt.float8e4)
    m = n // c
    et = e // 128
    nt = n // 128
    X = np.ascontiguousarray(X, dtype=np.float32)
    wqt = np.ascontiguousarray(np.asarray(Wq, dtype=np.float32).T.astype(f8))
    wkt = np.ascontiguousarray(np.asarray(Wk, dtype=np.float32).T.astype(f8))
    bq = np.ascontiguousarray(Wq0, dtype=np.float32).reshape(et, 128)
    bk = np.ascontiguousarray(Wk0, dtype=np.float32).reshape(et, 128)
    # xp[e_t, p, n_t, cc] = X[n_t*128 + p, e_t*128 + cc], fp8
    xp = np.ascontiguousarray(
        X.astype(f8).reshape(nt, 128, et, 128).transpose(2, 1, 0, 3)
    )
    # exact colsum of X for the centering correction, pre-divided by N
    sx = np.ascontiguousarray(
        (X.astype(np.float64).sum(axis=0) / n).astype(np.float32).reshape(et, 128)
    )
    in_maps = []
    for cc in range(c):
        xt_c = np.ascontiguousarray(X[cc * m:(cc + 1) * m].T.astype(f8))
        in_maps.append(
            {"xt": xt_c, "wqt": wqt, "wkt": wkt, "bq": bq, "bk": bk,
             "xp": xp, "sx": sx}
        )
    return in_maps


def _run(X, Wk, Wq, Wk0, Wq0, trace=False, n=N, e=E, c=C):
    from concourse import bass_utils

    key = (n, e, c)
    if key not in _CACHE:
        _CACHE[key] = _build_program(n, e, c)
    nc = _CACHE[key]
    in_maps = _prep_inputs(X, Wk, Wq, Wk0, Wq0, n, e, c)
    res = bass_utils.run_bass_kernel_spmd(
        nc, in_maps, core_ids=list(range(c)), trace=trace
    )
    m = n // c
    Z = np.empty((n, e), dtype=np.float32)
    for cc in range(c):
        Z[cc * m:(cc + 1) * m, :] = res.results[cc]["zt"].T
    return Z, res


def kernel(X, Wk, Wq, Wk0, Wq0):
    Z, _ = _run(X, Wk, Wq, Wk0, Wq0)
    return Z



# revision 25
# speedup vs baseline: 1.1218x; 1.1218x over previous
"""TRN2 Bass/Tile kernel for the cosine-similarity attention block.

Reference math (fp32, single device):
    K = X @ Wk.T + Wk0 ; Q = X @ Wq.T + Wq0          # [N, E]
    Y = (Q @ K.T) / sqrt(max(|Q_m|^2 * |K_n|^2, eps)) # [N, N] cosine sims
    SM = softmax(Y, axis=0)                           # column softmax
    Z = SM @ X                                        # [N, E]

Distribution (8 cores, row-sharded): each core owns M = N/8 rows of Q /
output rows of Z. Everything heavy runs as fp8-e4m3 DoubleRow matmuls
(2 k-tiles per pass at 0.5 cyc/row; fp32 PSUM accumulation):

  Phase 0: K/Q projections land DIRECTLY as x4 fp8 (one activation per
    tile: scale=4, bias=4*b); row norms come from fp8 squares (16K^2)
    reduced over the partition axis with a DoubleRow ones-matmul. K
    ships straight off the projection -- its AllGather launches as soon
    as the last K tile lands -- while 1/(16|k_n|) travels as a tiny
    second fp16 AllGather folded into the exp's per-partition scale
    (XBAR DMA-transposed on the reader). Q pays the broadcast-normalize
    (1/|q| ones-matmul -> PSUM -> ScalarE copy to SBUF so the DVE muls
    avoid the PSUM access penalty), m-chunked and t-inner so phase 2's
    o-major matmul groups can start on the first chunk.
  Phase 2: Yt slice via DoubleRow (psy holds 16|k_n|*y), exp on ScalarE
    with scale=1/(16|k_n|) and accum_out producing per-column partial
    sums for free (fp16 ring buffer). Column sums AllReduce in 4 chunks
    of 16 n-tiles; chunks 0-2 fold inline on DVE, chunk 3's AR is
    LAUNCHED at the last exp but its folds are emitted only after
    phase 4's first two A-groups (see below) so the conservative
    cross-engine waits on phase 4's first Ldweights do not chain it
    behind the tail folds.
  Phase 3 folds: v = (exp(y) - cs/8192) * (2^17/cs), i.e.
    2^17*(SM - 1/8192), on DVE tensor_scalar (2x_2p mode: ~533ns/tile)
    into fp8 pair tiles. Centering before the fp8 quantization is what
    makes fp8 viable: softmax weights concentrate at 1/8192*(1 +- 0.12),
    so raw e4m3 would destroy the signal, while the centered residuals
    span binades. (GPSIMD is software-emulated on TRN2 -- never put
    bulk elementwise work there.)
  Phase 4: Zt = X^T-panels(fp8) @ V(fp8) via DoubleRow. Each t-panel
    accumulates as A (pairs 0..23, chunks 0-2 of V) + B (pairs 24..31,
    chunk 3) into one PSUM group; emission order A0 A1 [chunk-3 folds]
    B0 A2 B1 A3 ... so the in-order PE never waits on the AllReduce
    tail. Then Zt = psz/2^17 + Sx/8192 with Sx = colsum of X
    (host-exact), restoring the centering.

DMA discipline: every dma_start pays ~650ns HWDGE + ~900ns semaphore
latency regardless of size, so bulk tensors move as ONE descriptor-rich
DMA each ([128, t, m] tiles via rearrange). The first K-block/X^T-panel
DMAs are emitted immediately after the AllGather so the SP queue's
in-order head never parks them behind the norm-chain shipments.

A dummy [1,1] Exp right after the norm chain preloads the ScalarE
exp_and_others table (which also covers Identity/Copy) during idle
time; without it the 1283ns table load lands right before exp #0.

The repeat loop covers all phases (per-rep AG/AR buffers) so a
repeat-NEFF slope measures honest steady-state per-exec device time.
"""

from contextlib import ExitStack

import numpy as np

N, E, C = 8192, 1024, 8

_CACHE = {}


def _build_program(n=N, e=E, c=C, solo=False, repeat=1, upto=4):
    """Emit + compile the SPMD Bass program (one NEFF, all cores)."""
    import concourse.bacc as bacc
    import concourse.mybir as mybir
    import concourse.tile as tile

    F32 = mybir.dt.float32
    F16 = mybir.dt.float16
    F8 = mybir.dt.float8e4
    AF = mybir.ActivationFunctionType
    PM = mybir.MatmulPerfMode.DoubleRow
    ALU = mybir.AluOpType

    m = n // c          # rows per core
    et = e // 128       # e-tiles
    nt = n // 128       # n-tiles
    jt = m // 128       # n-tiles per core block
    mch = [(i, min(512, m - i)) for i in range(0, m, 512)]  # m chunks (<=512)
    CH = 16             # AR chunk size in n-tiles
    nch = nt // CH
    EB = 32             # fp16 exp ring size (tiles)
    S_V = 131072.0      # 2^17: fp8 scale for centered softmax weights
    rg = [list(range(c))]

    nc = bacc.Bacc("TRN2", target_bir_lowering=False, debug=False, num_devices=c)

    xt = nc.dram_tensor("xt", [e, m], F8, kind="ExternalInput")
    wqt = nc.dram_tensor("wqt", [e, e], F8, kind="ExternalInput")
    wkt = nc.dram_tensor("wkt", [e, e], F8, kind="ExternalInput")
    bq = nc.dram_tensor("bq", [et, 128], F32, kind="ExternalInput")  # 4*Wq0
    bk = nc.dram_tensor("bk", [et, 128], F32, kind="ExternalInput")  # 4*Wk0
    xp = nc.dram_tensor("xp", [et, 128, nt, 128], F8, kind="ExternalInput")
    sx = nc.dram_tensor("sx", [et, 128], F32, kind="ExternalInput")  # X colsum/8192
    zt = nc.dram_tensor("zt", [e, m], F16, kind="ExternalOutput")

    with ExitStack() as ctx:
        tc = ctx.enter_context(tile.TileContext(nc))

        dram = ctx.enter_context(tc.tile_pool(name="dram", bufs=1, space="DRAM"))

        consts = ctx.enter_context(tc.tile_pool(name="consts", bufs=1))
        ones_k8 = consts.tile([128, 1], F16)
        ones1_m = consts.tile([1, 128], F16)
        nc.vector.memset(ones_k8, 1.0)
        nc.vector.memset(ones1_m, 1.0)
        bias_q = consts.tile([128, et], F32)
        bias_k = consts.tile([128, et], F32)
        nc.sync.dma_start(bias_q, bq.ap().rearrange("t p -> p t"))
        nc.sync.dma_start(bias_k, bk.ap().rearrange("t p -> p t"))
        sxb = consts.tile([128, et], F32)
        nc.sync.dma_start(sxb, sx.ap().rearrange("t p -> p t"))
        eps1 = consts.tile([1, 1], F32)
        nc.vector.memset(eps1, 1e-6)
        eps256 = consts.tile([1, 1], F32)
        nc.vector.memset(eps256, 2.56e-4)  # 256 * 1e-6, for 16*sqrt(x+eps)
        dume = consts.tile([1, 1], F32)
        sck16 = consts.tile([128, nt], F16)  # XBAR-transposed 1/(16|k_n|)
        sck32 = consts.tile([128, nt], F32)  # fp32 copy (scale APs must be fp32)
        colsum = consts.tile([128, nt], F32)
        cs_full = consts.tile([128, nt], F32)
        rec_cs = consts.tile([128, nt], F32)
        sfac = consts.tile([128, nt], F32)   # 2^17 / cs
        tfac = consts.tile([128, nt], F32)   # cs / 8192

        # persistent across reps: fp8 Qn^T [128, et, m] (pair slices via AP),
        # fp8 centered-SM tiles in DoubleRow pair layout [128, 2, m], and the
        # phase-0 working set (persistent tags let the next rep's input DMAs
        # start as soon as the previous rep's last reader is done, instead of
        # waiting for a whole pool region to free).
        qn_pool = ctx.enter_context(tc.tile_pool(name="qn", bufs=1))
        qn8 = qn_pool.tile([128, et, m], F8, tag="qn8", name="qn8")
        et_pool = ctx.enter_context(tc.tile_pool(name="etp", bufs=1))
        ets8 = [et_pool.tile([128, 2, m], F8, tag=f"e8_{i}", name=f"e8_{i}")
                for i in range(nt // 2)]
        p0 = ctx.enter_context(tc.tile_pool(name="p0", bufs=1))
        wk8 = p0.tile([128, et, e], F8, tag="wk8", name="wk8")
        wq8 = p0.tile([128, et, e], F8, tag="wq8", name="wq8")

        # phase-2/4 pools live at rep scope so the K-block / X^T-panel DMAs
        # can be emitted right after the AllGather (SP queue is in-order).
        for rep in range(repeat):
            # per-rep AG buffers: a Shared DRAM buffer may have only one
            # writing instruction, so each rep's AllGather gets its own.
            ag_in = dram.tile([e, m], F8, tag=f"agi{rep}", name=f"agi{rep}")
            ag_out = dram.tile([c, e, m], F8, addr_space="Shared",
                               tag=f"ago{rep}", name=f"ago{rep}")
            rk_in = dram.tile([m], F16, tag=f"rki{rep}", name=f"rki{rep}")
            rk_out = dram.tile([c, m], F16, addr_space="Shared",
                               tag=f"rko{rep}", name=f"rko{rep}")

            with (
                tc.tile_pool(name="xpp", bufs=4) as xp_pool,
                tc.tile_pool(name="zsb", bufs=2) as z_pool,
                tc.tile_pool(name="kp", bufs=2) as kp_pool,
            ):
                xq_pre = {}

                def load_xq(t):
                    xq = xp_pool.tile([128, nt, 128], F8, tag="xq", name="xq")
                    nc.sync.dma_start(xq, xp.ap()[t, :, :, :])
                    xq_pre[t] = xq
                    return xq

                def load_kp(cc):
                    kp = kp_pool.tile([128, et, m], F8, tag="kp", name="kp")
                    src = (ag_in[:, :] if solo else ag_out[cc])
                    nc.sync.dma_start(
                        kp, src.rearrange("(t p) m -> p t m", p=128))
                    return kp

                # ------------- Phase 0: projections + row norms ------------
                with (
                    tc.tile_pool(name="p0t", bufs=1) as p0t,
                    tc.tile_pool(name="psp", bufs=2, space="PSUM") as psp,
                    tc.tile_pool(name="pss", bufs=1, space="PSUM") as pss,
                ):
                    x8 = p0t.tile([128, et, m], F8, tag="x8", name="x8")
                    # half-granular head loads so the first projection
                    # matmuls start after ~1/2 of the weight/X bytes land.
                    for lo, hi in ((0, 512), (512, e)):
                        nc.sync.dma_start(
                            wk8[:, :, lo:hi],
                            wkt.ap()[:, lo:hi].rearrange(
                                "(t p) f -> p t f", p=128))
                        nc.sync.dma_start(
                            x8[:, :, lo:hi],
                            xt.ap()[:, lo:hi].rearrange(
                                "(t p) f -> p t f", p=128))
                    nc.sync.dma_start(
                        wq8, wqt.ap().rearrange("(t p) f -> p t f", p=128))

                    # fp8 x4 projections (one Act op per tile) + fp8 squares
                    # (16*K^2) in DoubleRow pair layout for the norm reduce.
                    # F16: the squares are (4K)^2 <= ~450, which
                    # overflows fp8-e4m3 in the 5-sigma tail. One shared set:
                    # d_ps consumes K's squares before Q's are produced.
                    sqs = [p0t.tile([128, 2, m], F16, tag=f"sq{p}",
                                    name=f"sq{p}") for p in range(et // 2)]
                    q8s = [p0t.tile([128, m], F8, tag=f"q8_{t}", name=f"q8_{t}")
                           for t in range(et)]
                    kst = [p0t.tile([128, m], F8, tag=f"kst{t % 2}",
                                    name=f"kst{t % 2}") for t in range(2)]

                    def proj_mm(wbuf, bias_sb, out8, post_t):
                        for t in range(et):
                            ps = psp.tile([128, m], F32, tag="pp",
                                          name="proj_ps")
                            for o, w in mch:
                                for sp in range(et // 2):
                                    lw = wbuf[:, 2 * sp:2 * sp + 2,
                                              t * 128:(t + 1) * 128]
                                    nc.tensor.matmul(
                                        ps[:, o:o + w],
                                        lw,
                                        x8[:, 2 * sp:2 * sp + 2, o:o + w],
                                        start=(sp == 0),
                                        stop=(sp == et // 2 - 1),
                                        perf_mode=PM,
                                    )
                            dst = out8(t)
                            nc.scalar.activation(dst, ps, AF.Identity,
                                                 scale=4.0,
                                                 bias=bias_sb[:, t:t + 1])
                            sq = sqs[t // 2][:, t % 2, :]
                            for o, w in mch:
                                nc.vector.tensor_mul(sq[:, o:o + w],
                                                     dst[:, o:o + w],
                                                     dst[:, o:o + w])
                            post_t(t, dst)

                    def d_ps_reduce():
                        # ones-matmul partition reduce: d_ps[m] = sum_e 16K^2
                        # (walrus rejects DoubleRow with narrow stationary
                        # columns, so this stays a plain 1 cyc/row matmul)
                        d_ps = pss.tile([1, m], F32, tag="dps", name="d_ps")
                        for o, w in mch:
                            for t in range(et):
                                nc.tensor.matmul(
                                    d_ps[0:1, o:o + w],
                                    ones_k8,
                                    sqs[t // 2][:, t % 2, o:o + w],
                                    start=(t == 0),
                                    stop=(t == et - 1),
                                )
                        return d_ps

                    # K ships straight off the projection (x4 fp8) --
                    # 1/(16|k_n|) folds into the exp scale instead, so the
                    # AllGather launches as soon as the last K tile lands.
                    def k_ship(t, dst):
                        nc.sync.dma_start(ag_in[t * 128:(t + 1) * 128, :], dst)

                    proj_mm(wk8, bias_k, lambda t: kst[t % 2],
                            k_ship)
                    if not solo:
                        nc.gpsimd.collective_compute(
                            "AllGather",
                            mybir.AluOpType.bypass,
                            replica_groups=rg,
                            ins=[ag_in.opt()],
                            outs=[ag_out.opt()],
                        )
                    # prefetch phase 2/4 bulk inputs now: the SP queue
                    # is in-order, and everything later in phase 0 is tiny.
                    kp0 = load_kp(0)
                    if upto >= 4:
                        load_xq(0)
                    kp1 = load_kp(1)
                    if upto >= 4:
                        load_xq(1)

                    # rk = 1/(16|k|): d_psk holds 16|k|^2, so 16|k| =
                    # sqrt(16*x + 256*eps) in one activation.
                    d_psk = d_ps_reduce()
                    dsqk = p0t.tile([1, m], F32, tag="dsqk", name="dsqk")
                    nc.scalar.activation(dsqk, d_psk, AF.Sqrt, scale=16.0,
                                         bias=eps256[0:1, 0:1])
                    rkf = p0t.tile([1, m], F32, tag="rkf", name="rkf")
                    nc.vector.reciprocal(rkf, dsqk)
                    rk16 = p0t.tile([1, m], F16, tag="rk16", name="rk16")
                    nc.vector.tensor_copy(rk16, rkf)
                    nc.gpsimd.dma_start(rk_in[:], rk16[0:1, :])
                    if not solo:
                        nc.gpsimd.collective_compute(
                            "AllGather",
                            mybir.AluOpType.bypass,
                            replica_groups=rg,
                            ins=[rk_in.opt()],
                            outs=[rk_out.opt()],
                        )
                    proj_mm(wq8, bias_q, lambda t: q8s[t],
                            lambda t, d: None)

                    # q norm chain: d_psq holds 16|q|^2 -> |q| =
                    # sqrt(x/16 + eps); m-chunked, t-inner, so phase 2's
                    # o-major groups start after the first chunk's 8 muls.
                    d_psq = d_ps_reduce()
                    bc_ps = pss.tile([128, m], F32, tag="bc", name="bc_ps")
                    bc_sb = p0t.tile([128, m], F32, tag="bcs", name="bc_sb")
                    dsq = p0t.tile([1, m], F32, tag="dsq", name="dsq")
                    rnorm = p0t.tile([1, m], F32, tag="rn", name="rnorm")
                    rn16 = p0t.tile([1, m], F16, tag="rn16", name="rn16")
                    for o, w in mch:
                        nc.scalar.activation(dsq[0:1, o:o + w],
                                             d_psq[0:1, o:o + w],
                                             AF.Sqrt, scale=0.0625,
                                             bias=eps1[0:1, 0:1])
                        nc.vector.reciprocal(rnorm[0:1, o:o + w],
                                             dsq[0:1, o:o + w])
                        nc.vector.tensor_copy(rn16[0:1, o:o + w],
                                              rnorm[0:1, o:o + w])
                        nc.tensor.matmul(
                            bc_ps[:, o:o + w],
                            ones1_m,
                            rn16[0:1, o:o + w],
                        )
                        # PSUM -> SBUF via ScalarE so the DVE muls skip the
                        # PSUM access penalty (~226ns/op). The normalize muls
                        # gate phase 2's start and DVE is the phase-0 floor,
                        # so 3 of 8 tiles per chunk go to the otherwise-idle
                        # GPSIMD queue (1.0us/op at 0.42 eff vs 0.59 on DVE;
                        # both finish in ~6us instead of 9.5 DVE-serial).
                        # t0/t1 stay on DVE: the first psy group reads them.
                        nc.scalar.activation(bc_sb[:, o:o + w],
                                             bc_ps[:, o:o + w], AF.Copy)
                        for t in range(et):
                            eng = nc.gpsimd if t in (3, 5, 7) else nc.vector
                            eng.tensor_mul(qn8[:, t, o:o + w],
                                           bc_sb[:, o:o + w],
                                           q8s[t][:, o:o + w])
                    # preload the exp_and_others ScalarE table (covers
                    # Identity/Copy too) in this idle window instead of
                    # right before exp #0.
                    nc.scalar.activation(dume, eps1, AF.Exp)

                if upto < 2:
                    continue

                # exp scale tile sck16[p, i] = 1/(16|k_{i*128+p}|): transpose
                # the gathered [c*jt, 128] rk rows into partition-major via
                # the XBAR DMA (16-bit only).
                if solo:
                    nc.vector.memset(sck32, 0.0625)
                else:
                    nc.scalar.dma_start_transpose(
                        sck16,
                        rk_out[:, :].rearrange("c (t p) -> (c t) p", p=128),
                    )
                    nc.vector.tensor_copy(sck32, sck16)

                with tc.tile_pool(name="psz", bufs=1,
                                  space="PSUM") as psz_pool:
                 with (
                    tc.tile_pool(name="eA", bufs=1) as eA_pool,
                    tc.tile_pool(name="psy", bufs=2, space="PSUM") as psy_pool,
                 ):
                    etsA = [eA_pool.tile([128, m], F16, tag=f"eA{i}",
                                         name=f"eA{i}") for i in range(EB)]

                    # ---- Phase 3 helpers: AR one chunk of CH n-tiles, then
                    # fold v = (exp(y) - cs/8192) * (2^17/cs) into fp8 pairs.
                    ar_bufs = {}

                    def ar_launch(k):
                        sl = slice(CH * k, CH * (k + 1))
                        ar_in = dram.tile([128, CH], F32, tag=f"ari{rep}_{k}",
                                          name=f"ari{rep}_{k}")
                        ar_out = dram.tile([128, CH], F32, addr_space="Shared",
                                           tag=f"aro{rep}_{k}",
                                           name=f"aro{rep}_{k}")
                        ar_bufs[k] = (ar_in, ar_out)
                        eng = nc.sync if k == nch - 1 else nc.gpsimd
                        eng.dma_start(ar_in, colsum[:, sl])
                        if not solo:
                            nc.gpsimd.collective_compute(
                                "AllReduce",
                                mybir.AluOpType.add,
                                replica_groups=rg,
                                ins=[ar_in.opt()],
                                outs=[ar_out.opt()],
                            )

                    def ar_fold(k, act_share=0):
                        sl = slice(CH * k, CH * (k + 1))
                        ar_in, ar_out = ar_bufs[k]
                        eng = nc.sync if k == nch - 1 else nc.gpsimd
                        eng.dma_start(cs_full[:, sl],
                                      ar_in if solo else ar_out)
                        nc.vector.reciprocal(rec_cs[:, sl], cs_full[:, sl])
                        nc.vector.tensor_scalar_mul(sfac[:, sl],
                                                    rec_cs[:, sl], S_V)
                        nc.vector.tensor_scalar_mul(tfac[:, sl],
                                                    cs_full[:, sl],
                                                    1.0 / 8192.0)
                        for idx, i in enumerate(range(CH * k, CH * (k + 1))):
                            dst = ets8[i // 2][:, i % 2, :]
                            src_t = etsA[i % EB]
                            if idx % 3 == 1 and act_share:
                                nc.scalar.activation(dst, src_t, AF.Copy,
                                                     scale=sfac[:, i:i + 1],
                                                     bias=-16.0)
                            else:
                                nc.vector.tensor_scalar(
                                    dst, src_t, tfac[:, i:i + 1],
                                    sfac[:, i:i + 1],
                                    ALU.subtract, ALU.mult)

                    # t0/t1's phase-4 accumulations interleave into phase 2's
                    # PE slack (the exps are the bottleneck there), one chunk
                    # BEHIND the AllReduce so a slow collective never stalls
                    # the in-order PE queue.
                    psz_int = {}

                    def int_mms(t, pr_lo, pr_hi, start=False, stop=False):
                        psz = psz_int[t]
                        xq = xq_pre[t]
                        for pair in range(pr_lo, pr_hi):
                            lw = xq[:, 2 * pair:2 * pair + 2, :]
                            for o, w in mch:
                                nc.tensor.matmul(
                                    psz[:, o:o + w], lw,
                                    ets8[pair][:, :, o:o + w],
                                    start=(start and pair == pr_lo),
                                    stop=(stop and pair == pr_hi - 1),
                                    perf_mode=PM,
                                )

                    # ------------- Phase 2: Yt via fp8 DoubleRow -----------
                    for cc in range(c):
                        kp = (kp0, kp1)[cc] if cc < 2 else load_kp(cc)
                        for j in range(jt):
                            i = cc * jt + j
                            psy = psy_pool.tile([128, m], F32, tag="py",
                                                name="psy")
                            for o, w in mch:
                                for sp in range(et // 2):
                                    lw = kp[:, 2 * sp:2 * sp + 2,
                                            j * 128:(j + 1) * 128]
                                    nc.tensor.matmul(
                                        psy[:, o:o + w],
                                        lw,
                                        qn8[:, 2 * sp:2 * sp + 2, o:o + w],
                                        start=(sp == 0),
                                        stop=(sp == et // 2 - 1),
                                        perf_mode=PM,
                                    )
                            nc.scalar.activation(
                                etsA[i % EB], psy, AF.Exp,
                                scale=sck32[:, i:i + 1],
                                accum_out=colsum[:, i:i + 1],
                            )
                            if i % CH == CH - 1 and upto >= 3:
                                k = i // CH
                                ar_launch(k)
                                if upto >= 4 and k >= 1:
                                    for t in (0, 1):
                                        if k == 1:
                                            psz_int[t] = psz_pool.tile(
                                                [128, m], F32, tag=f"pz{t}",
                                                name=f"pz{t}")
                                        int_mms(t, CH // 2 * (k - 1),
                                                CH // 2 * k, start=(k == 1))
                                ar_fold(k, act_share=(k == nch - 1))
                                if upto >= 4 and k in (1, 2):
                                    load_xq(k + 1)

                 # -------- Phase 4: Zt = X^T @ V (fp8 DoubleRow) ----------
                 # psy's banks are free now; a second PSUM pool lets t2/t3
                 # A-groups run while the AllReduce tail folds chunk 3.
                 if upto >= 4:
                  with tc.tile_pool(name="pszB", bufs=1,
                                    space="PSUM") as pszB_pool:
                    PA = (nt // 2) - CH // 2
                    pszB_t = {}

                    def full_A(t):
                        psz = pszB_pool.tile([128, m], F32, tag=f"pzB{t % 2}",
                                             name=f"pzB{t % 2}")
                        pszB_t[t] = psz
                        xq = xq_pre[t]
                        for pair in range(PA):
                            lw = xq[:, 2 * pair:2 * pair + 2, :]
                            for o, w in mch:
                                nc.tensor.matmul(
                                    psz[:, o:o + w], lw,
                                    ets8[pair][:, :, o:o + w],
                                    start=(pair == 0), stop=False,
                                    perf_mode=PM,
                                )

                    def full_B(t):
                        psz = pszB_t[t]
                        xq = xq_pre[t]
                        for pair in range(PA, nt // 2):
                            lw = xq[:, 2 * pair:2 * pair + 2, :]
                            for o, w in mch:
                                nc.tensor.matmul(
                                    psz[:, o:o + w], lw,
                                    ets8[pair][:, :, o:o + w],
                                    start=False, stop=(pair == nt // 2 - 1),
                                    perf_mode=PM,
                                )
                        return pszB_t.pop(t)

                    def drain(t, psz):
                        xq_pre.pop(t, None)
                        zsb = z_pool.tile([128, m], F16, tag="zt", name="zsb")
                        nc.scalar.activation(zsb, psz, AF.Identity,
                                             scale=1.0 / S_V,
                                             bias=sxb[:, t:t + 1])
                        nc.sync.dma_start(zt.ap()[t * 128:(t + 1) * 128, :],
                                          zsb)

                    full_A(2)
                    full_A(3)
                    for t in (0, 1):
                        int_mms(t, PA, nt // 2, stop=True)
                        drain(t, psz_int.pop(t))
                        load_xq(t + 4)
                    drain(2, full_B(2))
                    load_xq(6)
                    drain(3, full_B(3))
                    load_xq(7)
                    for t in range(4, et):
                        full_A(t)
                        drain(t, full_B(t))

    nc.compile()
    return nc


def _prep_inputs(X, Wk, Wq, Wk0, Wq0, n=N, e=E, c=C):
    """Host-side sharding/layout prep. Returns per-core input maps."""
    import concourse.mybir as mybir

    f8 = mybir.dt.np(mybir.dt.float8e4)
    m = n // c
    et = e // 128
    nt = n // 128
    X = np.ascontiguousarray(X, dtype=np.float32)
    wqt = np.ascontiguousarray(np.asarray(Wq, dtype=np.float32).T.astype(f8))
    wkt = np.ascontiguousarray(np.asarray(Wk, dtype=np.float32).T.astype(f8))
    # device projections emit x4 fp8, so biases ship pre-scaled by 4
    bq = np.ascontiguousarray(4.0 * np.asarray(Wq0, dtype=np.float32)).reshape(et, 128)
    bk = np.ascontiguousarray(4.0 * np.asarray(Wk0, dtype=np.float32)).reshape(et, 128)
    # xp[e_t, p, n_t, cc] = X[n_t*128 + p, e_t*128 + cc], fp8
    xp = np.ascontiguousarray(
        X.astype(f8).reshape(nt, 128, et, 128).transpose(2, 1, 0, 3)
    )
    # exact colsum of X for the centering correction, pre-divided by N
    sx = np.ascontiguousarray(
        (X.astype(np.float64).sum(axis=0) / n).astype(np.float32).reshape(et, 128)
    )
    in_maps = []
    for cc in range(c):
        xt_c = np.ascontiguousarray(X[cc * m:(cc + 1) * m].T.astype(f8))
        in_maps.append(
            {"xt": xt_c, "wqt": wqt, "wkt": wkt, "bq": bq, "bk": bk,
             "xp": xp, "sx": sx}
        )
    return in_maps


def _run(X, Wk, Wq, Wk0, Wq0, trace=False, n=N, e=E, c=C):
    from concourse import bass_utils

    key = (n, e, c)
    if key not in _CACHE:
        _CACHE[key] = _build_program(n, e, c)
    nc = _CACHE[key]
    in_maps = _prep_inputs(X, Wk, Wq, Wk0, Wq0, n, e, c)
    res = bass_utils.run_bass_kernel_spmd(
        nc, in_maps, core_ids=list(range(c)), trace=trace
    )
    m = n // c
    Z = np.empty((n, e), dtype=np.float32)
    for cc in range(c):
        Z[cc * m:(cc + 1) * m, :] = res.results[cc]["zt"].T
    return Z, res


def kernel(X, Wk, Wq, Wk0, Wq0):
    Z, _ = _run(X, Wk, Wq, Wk0, Wq0)
    return Z
